# revision 2
# baseline (speedup 1.0000x reference)
"""AutoCompleteDecoderModel loss kernel for trn2, 8-core data parallel.

B=128, Lc=512, Le=512, H=512, V=128. Encoder LSTM + attention LSTM
decoder (teacher forcing) + masked CE -> scalar f32 loss.

Sharding: pure data parallel, batch 16 rows/core across 8 NeuronCores.
Device program (Bass/Tile): feature dims on SBUF partitions, batch on the
free dim; bf16 stationary-weight matmuls (N=16 rhs); attention einsums use
enc_hs tiles as the stationary operand with N=1 rhs so both outputs land
in the transposed layout the next stage needs; softmax normalization is
deferred until after the attention-weighted sum. Biases are zero in this
model and omitted. Per-core output is a (1,16) row of per-sample NLL
sums; the host divides by the mask count.

Self-contained: only imports installed packages (concourse, jax, numpy).
"""
import contextlib
import numpy as np
import ml_dtypes

import bass_rust
import concourse.bass as bass
import concourse.mybir as mybir
import concourse.tile as tile
from concourse.vector_clock import ScopedClock

F32 = mybir.dt.float32
BF16 = mybir.dt.bfloat16
AF = mybir.ActivationFunctionType
BF = ml_dtypes.bfloat16

B, Lc, Le, H, V = 128, 512, 512, 512, 128
PAD_IDX = 0
M = 8
Bs = B // M      # 16 rows per core
NH = H // 128    # 4
NG = 4 * H // 128  # 16
T_ENC = Lc
T_DEC = Le - 1   # 511

# ---------------------------------------------------------------------------
# Workaround: this walrus build accepts only ONE sync-wait per instruction;
# Tile's scheduler emits up to 3. Spill extra waits onto same-engine NoOps.
# ---------------------------------------------------------------------------
_MAX_WAITS = 1
_orig_lower = tile.TileContext._lower_ordered_insts
_uid = [0]


def _split_inst_waits(inst):
    si = inst.sync_info
    if si is None:
        return None
    waits = list(si.on_wait)
    if len(waits) <= _MAX_WAITS:
        return None
    spill, keep = waits[:-1], [waits[-1]]
    nops = []
    for w in spill:
        nop = mybir.InstNoOp()
        nop.engine = inst.engine
        _uid[0] += 1
        nop.name = f"{inst.name}_sw{_uid[0]}"
        nop.sync_info = bass_rust.SyncInfo(on_wait=[w], on_update=[])
        nops.append(nop)
    inst.sync_info = bass_rust.SyncInfo(on_wait=keep, on_update=list(si.on_update))
    return nops


def _patched_lower_ordered_insts(self, ordered):
    for bb_name, insts in ordered.items():
        out = []
        for inst in insts:
            nops = _split_inst_waits(inst)
            if nops:
                out.extend(nops)
            out.append(inst)
        if len(out) != len(insts):
            insts[:] = out
    return _orig_lower(self, ordered)


def _patched_drain_and_barrier(self, tick_clock, wait_clock):
    nc = self.nc
    drain_inst = nc.sync.drain()
    wait_clock.add_sem_waits(
        drain_inst.ins, ScopedClock({None: tick_clock.global_clock}))
    si = drain_inst.ins.sync_info
    waits = list(si.on_wait) if si is not None else []
    updates = list(si.on_update) if si is not None else []
    if len(waits) > _MAX_WAITS:
        chunks = [waits[i:i + _MAX_WAITS] for i in range(0, len(waits), _MAX_WAITS)]
        drain_inst.ins.sync_info = bass_rust.SyncInfo(on_wait=chunks[0], on_update=[])
        last = drain_inst
        for ch in chunks[1:]:
            last = nc.sync.drain()
            last.ins.sync_info = bass_rust.SyncInfo(on_wait=ch, on_update=[])
        if updates:
            lsi = last.ins.sync_info
            last.ins.sync_info = bass_rust.SyncInfo(
                on_wait=list(lsi.on_wait) if lsi else [], on_update=updates)
    nc.all_engine_barrier()
    assert self.sems is not None
    popped = nc._tile_sem_poison_stack.pop()
    assert popped is self._sem_poison
    nc.clear_and_free_semaphores(list(self.sems.allocated().values()))
    nc.all_engine_barrier()


tile.TileContext._drain_and_barrier = _patched_drain_and_barrier
tile.TileContext._lower_ordered_insts = _patched_lower_ordered_insts


# ---------------------------------------------------------------------------
# Device program
# ---------------------------------------------------------------------------

def build_program(T_enc, T_dec):
    n_lc = T_enc // 128
    n_eg = T_enc // 8
    nc = bass.Bass()

    d_ct = nc.dram_tensor("ct", [T_enc, V, Bs], BF16, kind="ExternalInput")
    d_esb = nc.dram_tensor("esb", [V, T_dec * Bs], BF16, kind="ExternalInput")
    d_oh = nc.dram_tensor("oh", [V, T_dec * Bs], BF16, kind="ExternalInput")
    d_cem = nc.dram_tensor("cem", [1, T_dec * Bs], F32, kind="ExternalInput")
    d_amask = nc.dram_tensor("amask", [128, Bs * n_lc], F32, kind="ExternalInput")
    d_ewih = nc.dram_tensor("ewih", [V, 4 * H], BF16, kind="ExternalInput")
    d_ewhh = nc.dram_tensor("ewhh", [128, NH, 4 * H], BF16, kind="ExternalInput")
    d_wihe = nc.dram_tensor("wihe", [V, 4 * H], BF16, kind="ExternalInput")
    d_wihv = nc.dram_tensor("wihv", [128, NH, 4 * H], BF16, kind="ExternalInput")
    d_whh = nc.dram_tensor("whh", [128, NH, 4 * H], BF16, kind="ExternalInput")
    d_att = nc.dram_tensor("att", [128, NH, H], BF16, kind="ExternalInput")
    d_out = nc.dram_tensor("outw", [128, 2 * NH, H], BF16, kind="ExternalInput")
    d_voc = nc.dram_tensor("voc", [128, NH, V], BF16, kind="ExternalInput")
    d_encB = nc.dram_tensor("encB_scratch", [T_enc, NH, Bs, 128], BF16,
                            kind="Internal")
    d_nll = nc.dram_tensor("nll", [1, Bs], F32, kind="ExternalOutput")

    with tile.TileContext(nc) as tc, contextlib.ExitStack() as ctx:
        per = ctx.enter_context(tc.tile_pool(name="per", bufs=1))
        wk = ctx.enter_context(tc.tile_pool(name="wk", bufs=2))
        io = ctx.enter_context(tc.tile_pool(name="io", bufs=3))
        ps = ctx.enter_context(tc.tile_pool(name="ps", bufs=1, space="PSUM"))
        pss = ctx.enter_context(tc.tile_pool(name="pss", bufs=2, space="PSUM"))
        psg = ctx.enter_context(tc.tile_pool(name="psg", bufs=2, space="PSUM"))

        encA = per.tile([128, NH, Bs, T_enc], BF16)
        amask = per.tile([128, Bs * n_lc], F32)
        ones_bf = per.tile([128, 1], BF16)
        ones_f32 = per.tile([128, 1], F32)
        ones_row = per.tile([1, 128], F32)
        h_cur = per.tile([128, NH * Bs], BF16)
        c_st = per.tile([128, NH * Bs], F32)
        vprev = per.tile([128, NH * Bs], BF16)
        nll_acc = per.tile([1, Bs], F32)

        nc.sync.dma_start(out=amask, in_=d_amask[:, :])
        nc.vector.memset(ones_bf, 1.0)
        nc.vector.memset(ones_f32, 1.0)
        nc.vector.memset(ones_row, 1.0)
        nc.vector.memset(h_cur, 0.0)
        nc.vector.memset(c_st, 0.0)
        nc.vector.memset(vprev, 0.0)
        nc.vector.memset(nll_acc, 0.0)

        SB = NH * Bs  # 64

        def lstm_tail(psum_g):
            sig_i = wk.tile([128, SB], F32, tag="sig_i")
            sig_f = wk.tile([128, SB], F32, tag="sig_f")
            tanh_g = wk.tile([128, SB], F32, tag="tanh_g")
            sig_o = wk.tile([128, SB], F32, tag="sig_o")
            nc.scalar.activation(out=sig_i, in_=psum_g[:, 0 * SB:1 * SB], func=AF.Sigmoid)
            nc.scalar.activation(out=sig_f, in_=psum_g[:, 1 * SB:2 * SB], func=AF.Sigmoid)
            nc.scalar.activation(out=tanh_g, in_=psum_g[:, 2 * SB:3 * SB], func=AF.Tanh)
            nc.scalar.activation(out=sig_o, in_=psum_g[:, 3 * SB:4 * SB], func=AF.Sigmoid)
            t1 = wk.tile([128, SB], F32, tag="t1")
            nc.vector.tensor_mul(t1, sig_i, tanh_g)
            nc.vector.tensor_mul(c_st, sig_f, c_st)
            nc.vector.tensor_add(c_st, c_st, t1)
            tanh_c = wk.tile([128, SB], F32, tag="tanh_c")
            nc.scalar.activation(out=tanh_c, in_=c_st, func=AF.Tanh)
            return sig_o, tanh_c

        # ---------------- encoder ----------------
        with tc.tile_pool(name="encw", bufs=1) as encw:
            ewih = encw.tile([V, 4 * H], BF16)
            ewhh = encw.tile([128, NH, 4 * H], BF16)
            nc.sync.dma_start(out=ewih, in_=d_ewih[:, :])
            nc.sync.dma_start(out=ewhh, in_=d_ewhh[:, :, :])

            with tc.For_i(0, n_eg) as g:
                cchunk = io.tile([V, 8, Bs], BF16, tag="cchunk")
                nc.scalar.dma_start(
                    out=cchunk,
                    in_=d_ct[bass.ts(g, 8), :, :].rearrange("t v b -> v t b"))
                stage = wk.tile([128, 8 * SB], BF16, tag="stage")
                for j in range(8):
                    psum_g = psg.tile([128, NG * Bs], F32, tag="g")
                    for m in range(NG):
                        o_ap = psum_g[:, m * Bs:(m + 1) * Bs]
                        nc.tensor.matmul(o_ap, ewih[:, m * 128:(m + 1) * 128],
                                         cchunk[:, j, :], start=True, stop=False)
                        for k in range(NH):
                            nc.tensor.matmul(o_ap, ewhh[:, k, m * 128:(m + 1) * 128],
                                             h_cur[:, k * Bs:(k + 1) * Bs],
                                             start=False, stop=(k == NH - 1))
                    sig_o, tanh_c = lstm_tail(psum_g)
                    nc.vector.tensor_mul(stage[:, j * SB:(j + 1) * SB], sig_o, tanh_c)
                    nc.vector.tensor_copy(h_cur, stage[:, j * SB:(j + 1) * SB])
                src = stage.rearrange("p (t hc b) -> p hc b t", t=8, hc=NH, b=Bs)
                nc.vector.tensor_copy(encA[:, :, :, bass.ts(g, 8)], src)
                for q in range(4):
                    ttile = wk.tile([128, 128], BF16, tag="ttile")
                    nc.sync.dma_start(out=ttile, in_=stage[:, q * 128:(q + 1) * 128],
                                      transpose=True)
                    nc.sync.dma_start(
                        out=d_encB[bass.DynSlice(g * 8 + q * 2, 2), :, :, :],
                        in_=ttile)
            tc.strict_bb_all_engine_barrier()

        # ---------------- decoder ----------------
        with tc.tile_pool(name="decw", bufs=1) as decw:
            encB = decw.tile([128, n_lc, NH, Bs, 128], BF16)
            wihe = decw.tile([V, 4 * H], BF16)
            wihv = decw.tile([128, NH, 4 * H], BF16)
            whh = decw.tile([128, NH, 4 * H], BF16)
            attw = decw.tile([128, NH, H], BF16)
            outw = decw.tile([128, 2 * NH, H], BF16)
            vocw = decw.tile([128, NH, V], BF16)
            for lc in range(n_lc):
                nc.sync.dma_start(out=encB[:, lc, :, :, :],
                                  in_=d_encB[lc * 128:(lc + 1) * 128, :, :, :])
            for dst, srcd in ((wihe, d_wihe), (wihv, d_wihv), (whh, d_whh),
                              (attw, d_att), (outw, d_out), (vocw, d_voc)):
                nc.sync.dma_start(
                    out=dst, in_=srcd[tuple(slice(None) for _ in srcd.shape)])

            with tc.For_i(0, T_dec) as i:
                et = io.tile([V, Bs], BF16, tag="et")
                oht = io.tile([V, Bs], BF16, tag="oht")
                cemt = io.tile([1, Bs], F32, tag="cemt")
                nc.sync.dma_start(out=et, in_=d_esb[:, bass.ts(i, Bs)])
                nc.sync.dma_start(out=oht, in_=d_oh[:, bass.ts(i, Bs)])
                nc.sync.dma_start(out=cemt, in_=d_cem[:, bass.ts(i, Bs)])

                psum_g = psg.tile([128, NG * Bs], F32, tag="g")
                for m in range(NG):
                    o_ap = psum_g[:, m * Bs:(m + 1) * Bs]
                    nc.tensor.matmul(o_ap, wihe[:, m * 128:(m + 1) * 128], et,
                                     start=True, stop=False)
                    for k in range(NH):
                        nc.tensor.matmul(o_ap, wihv[:, k, m * 128:(m + 1) * 128],
                                         vprev[:, k * Bs:(k + 1) * Bs],
                                         start=False, stop=False)
                    for k in range(NH):
                        nc.tensor.matmul(o_ap, whh[:, k, m * 128:(m + 1) * 128],
                                         h_cur[:, k * Bs:(k + 1) * Bs],
                                         start=False, stop=(k == NH - 1))
                sig_o, tanh_c = lstm_tail(psum_g)
                nc.vector.tensor_mul(h_cur, sig_o, tanh_c)

                psum_q = ps.tile([128, SB], F32, tag="q")
                for m in range(NH):
                    for k in range(NH):
                        nc.tensor.matmul(psum_q[:, m * Bs:(m + 1) * Bs],
                                         attw[:, k, m * 128:(m + 1) * 128],
                                         h_cur[:, k * Bs:(k + 1) * Bs],
                                         start=(k == 0), stop=(k == NH - 1))
                q_bf = wk.tile([128, SB], BF16, tag="q_bf")
                nc.vector.tensor_copy(q_bf, psum_q)

                psum_s = ps.tile([128, Bs * n_lc], F32, tag="s")
                for b in range(Bs):
                    for lc in range(n_lc):
                        for hc in range(NH):
                            nc.tensor.matmul(
                                psum_s[:, b * n_lc + lc: b * n_lc + lc + 1],
                                encA[:, hc, b, lc * 128:(lc + 1) * 128],
                                q_bf[:, hc * Bs + b: hc * Bs + b + 1],
                                start=(hc == 0), stop=(hc == NH - 1))
                em_f = wk.tile([128, Bs * n_lc], F32, tag="em_f")
                nc.scalar.activation(out=em_f, in_=psum_s, func=AF.Exp)
                em_bf = wk.tile([128, Bs * n_lc], BF16, tag="em_bf")
                nc.vector.tensor_mul(em_bf, em_f, amask)

                psum_d = pss.tile([1, Bs * n_lc], F32, tag="small")
                nc.tensor.matmul(psum_d, ones_bf, em_bf, start=True, stop=True)
                den = wk.tile([1, Bs], F32, tag="den")
                nc.vector.reduce_sum(
                    den, psum_d.rearrange("p (b l) -> p b l", b=Bs),
                    axis=mybir.AxisListType.X)
                rcp = wk.tile([1, Bs], F32, tag="rcp")
                nc.vector.reciprocal(rcp, den)
                psum_r = pss.tile([128, Bs], F32, tag="small")
                nc.tensor.matmul(psum_r, ones_row, rcp, start=True, stop=True)
                rcps = wk.tile([128, Bs], F32, tag="rcps")
                nc.vector.tensor_copy(rcps, psum_r)

                psum_a = ps.tile([128, SB], F32, tag="a")
                for b in range(Bs):
                    for hc in range(NH):
                        for lc in range(n_lc):
                            nc.tensor.matmul(
                                psum_a[:, hc * Bs + b: hc * Bs + b + 1],
                                encB[:, lc, hc, b, :],
                                em_bf[:, b * n_lc + lc: b * n_lc + lc + 1],
                                start=(lc == 0), stop=(lc == n_lc - 1))
                attn_bf = wk.tile([128, SB], BF16, tag="attn_bf")
                _rap = rcps[:, :]
                rcp_b = bass.AP(tensor=_rap.tensor, offset=_rap.offset,
                                ap=[list(_rap.ap[0]), [0, NH], [1, Bs]])
                nc.vector.tensor_mul(
                    attn_bf, psum_a.rearrange("p (h b) -> p h b", h=NH), rcp_b)

                psum_v = ps.tile([128, SB], F32, tag="v")
                for m in range(NH):
                    o_ap = psum_v[:, m * Bs:(m + 1) * Bs]
                    for k in range(NH):
                        nc.tensor.matmul(o_ap, outw[:, k, m * 128:(m + 1) * 128],
                                         h_cur[:, k * Bs:(k + 1) * Bs],
                                         start=(k == 0), stop=False)
                    for k in range(NH):
                        nc.tensor.matmul(o_ap, outw[:, NH + k, m * 128:(m + 1) * 128],
                                         attn_bf[:, k * Bs:(k + 1) * Bs],
                                         start=False, stop=(k == NH - 1))
                tanh_v = wk.tile([128, SB], BF16, tag="tanh_v")
                nc.scalar.activation(out=tanh_v, in_=psum_v, func=AF.Tanh)
                nc.vector.tensor_copy(vprev, psum_v)

                psum_l = pss.tile([128, Bs], F32, tag="small")
                for k in range(NH):
                    nc.tensor.matmul(psum_l, vocw[:, k, :],
                                     tanh_v[:, k * Bs:(k + 1) * Bs],
                                     start=(k == 0), stop=(k == NH - 1))
                ce_buf = wk.tile([128, 2 * Bs], F32, tag="ce_buf")
                nc.scalar.activation(out=ce_buf[:, 0:Bs], in_=psum_l, func=AF.Exp)
                oh_f = wk.tile([128, Bs], F32, tag="oh_f")
                nc.vector.tensor_copy(oh_f, oht)
                nc.vector.tensor_mul(ce_buf[:, Bs:2 * Bs], psum_l, oh_f)
                psum_ce = pss.tile([1, 2 * Bs], F32, tag="small")
                nc.tensor.matmul(psum_ce, ones_f32, ce_buf, start=True, stop=True)
                lse = wk.tile([1, Bs], F32, tag="lse")
                nc.scalar.activation(out=lse, in_=psum_ce[:, 0:Bs], func=AF.Ln)
                dnll = wk.tile([1, Bs], F32, tag="dnll")
                nc.vector.tensor_sub(dnll, lse, psum_ce[:, Bs:2 * Bs])
                nc.vector.tensor_mul(dnll, dnll, cemt)
                nc.vector.tensor_add(nll_acc, nll_acc, dnll)

            tc.strict_bb_all_engine_barrier()
            nc.sync.dma_start(out=d_nll[:, :], in_=nll_acc)
    return nc


# ---------------------------------------------------------------------------
# Host-side prep
# ---------------------------------------------------------------------------

def _wT(w):
    I = w.shape[1]
    nk = I // 128
    wt = np.ascontiguousarray(w.T.reshape(nk, 128, w.shape[0]).transpose(1, 0, 2))
    return wt.astype(BF)


def prep_weights(inputs):
    w = {k: np.asarray(inputs[k], np.float32) for k in
         ("enc_Wih", "enc_Whh", "dec_Wih", "dec_Whh", "att_W", "out_W", "voc_W")}
    return {
        "ewih": np.ascontiguousarray(w["enc_Wih"].T).astype(BF),
        "ewhh": _wT(w["enc_Whh"]),
        "wihe": np.ascontiguousarray(w["dec_Wih"][:, :V].T).astype(BF),
        "wihv": _wT(w["dec_Wih"][:, V:]),
        "whh": _wT(w["dec_Whh"]),
        "att": _wT(w["att_W"]),
        "outw": _wT(w["out_W"]),
        "voc": _wT(w["voc_W"]),
    }


def prep_core_inputs(inputs, core, weights):
    T_enc, T_dec = T_ENC, T_DEC
    n_lc = T_enc // 128
    sl = slice(core * Bs, (core + 1) * Bs)
    C = np.asarray(inputs["C"], np.float32)[sl]
    C_pad = np.asarray(inputs["C_pad"])[sl]
    E = np.asarray(inputs["E"]).astype(np.int64)[sl]
    E_emb = np.asarray(inputs["E_emb"], np.float32)[sl]

    ct = np.ascontiguousarray(C.transpose(1, 2, 0)).astype(BF)
    ee = E_emb[:, :T_dec].transpose(2, 1, 0)
    esb = np.ascontiguousarray(ee.reshape(V, T_dec * Bs)).astype(BF)

    tgt = E[:, 1:T_dec + 1]
    ohm = np.zeros((V, T_dec, Bs), np.float32)
    tt, bb = np.meshgrid(np.arange(T_dec), np.arange(Bs), indexing="ij")
    ohm[tgt.T.ravel(), tt.ravel(), bb.ravel()] = 1.0
    oh = np.ascontiguousarray(ohm.reshape(V, T_dec * Bs)).astype(BF)

    cem = np.ascontiguousarray(
        (tgt.T != PAD_IDX).astype(np.float32).reshape(1, T_dec * Bs))

    am = (C_pad == 0).astype(np.float32).T
    amask = np.ascontiguousarray(
        am.reshape(n_lc, 128, Bs).transpose(1, 2, 0).reshape(128, Bs * n_lc))

    d = {"ct": ct, "esb": esb, "oh": oh, "cem": cem, "amask": amask}
    d.update(weights)
    return d


# ---------------------------------------------------------------------------
# Cached PJRT runner (mirrors concourse.bass2jax.run_bass_via_pjrt, but the
# jitted callable is built once and reused so repeat calls skip re-tracing).
# ---------------------------------------------------------------------------
_RUNNER = {}


def _get_runner():
    if _RUNNER:
        return _RUNNER
    import jax
    from jax.sharding import Mesh, PartitionSpec
    from jax.experimental.shard_map import shard_map
    from concourse.bass2jax import (_bass_exec_p, install_neuronx_cc_hook,
                                    partition_id_tensor)

    install_neuronx_cc_hook()
    nc = build_program(T_ENC, T_DEC)

    partition_name = nc.partition_id_tensor.name if nc.partition_id_tensor else None
    in_names, out_names, out_avals, zero_outs = [], [], [], []
    for alloc in nc.m.functions[0].allocations:
        if not isinstance(alloc, mybir.MemoryLocationSet):
            continue
        name = alloc.memorylocations[0].name
        if alloc.kind == "ExternalInput":
            if name != partition_name:
                in_names.append(name)
        elif alloc.kind == "ExternalOutput":
            shape = tuple(alloc.tensor_shape)
            dtype = mybir.dt.np(alloc.dtype)
            out_names.append(name)
            out_avals.append(jax.core.ShapedArray(shape, dtype))
            zero_outs.append(np.zeros(shape, dtype))
    n_params = len(in_names)
    all_names = list(in_names) + list(out_names)
    if partition_name is not None:
        all_names.append(partition_name)
    donate = tuple(range(n_params, n_params + len(out_names)))

    def _body(*args):
        operands = list(args)
        if partition_name is not None:
            operands.append(partition_id_tensor())
        outs = _bass_exec_p.bind(
            *operands,
            out_avals=tuple(out_avals),
            in_names=tuple(all_names),
            out_names=tuple(out_names),
            lowering_input_output_aliases=(),
            sim_require_finite=True,
            sim_require_nnan=True,
            nc=nc,
        )
        return tuple(outs)

    devices = jax.devices()[:M]
    mesh = Mesh(np.asarray(devices), ("core",))
    in_specs = (PartitionSpec("core"),) * (n_params + len(out_names))
    out_specs = (PartitionSpec("core"),) * len(out_names)
    sharded = jax.jit(
        shard_map(_body, mesh=mesh, in_specs=in_specs, out_specs=out_specs,
                  check_rep=False),
        donate_argnums=donate, keep_unused=True)

    _RUNNER.update(dict(fn=sharded, in_names=in_names, out_names=out_names,
                        out_avals=out_avals, zero_outs=zero_outs))
    return _RUNNER


def kernel(**inputs):
    r = _get_runner()
    weights = prep_weights(inputs)
    per_core = [prep_core_inputs(inputs, c, weights) for c in range(M)]
    concat_in = [np.concatenate([per_core[c][n] for c in range(M)], axis=0)
                 for n in r["in_names"]]
    concat_zeros = [np.zeros((M * z.shape[0],) + z.shape[1:], z.dtype)
                    for z in r["zero_outs"]]
    out_arrs = r["fn"](*concat_in, *concat_zeros)
    nll = np.asarray(out_arrs[0]).reshape(M, Bs)
    E = np.asarray(inputs["E"]).astype(np.int64)
    m_sum = float((E[:, 1:T_DEC + 1] != PAD_IDX).sum())
    return np.float32(float(nll.sum()) / max(m_sum, 1.0))
